# revision 1
# baseline (speedup 1.0000x reference)
"""Trainium2 Bass kernel for nn_B_Conv2d_ConvNN_K_N (retrieval_knn).

Data-parallel over 8 NeuronCores: 32 images/core, weights replicated.
Host-side prep reshapes inputs into device-friendly layouts (pixel-unshuffle,
lhsT weight layouts, wrapped gather-index tiles, per-position FC1 weights).
KNN score uses dots - sn/2 (order-equivalent to -squared-distance per token).
"""
import sys
if '/opt/trn_rl_repo' not in sys.path:
    sys.path.insert(0, '/opt/trn_rl_repo')

import numpy as np
import concourse.bacc as bacc
import concourse.mybir as mybir
from concourse.tile import TileContext
from concourse.bass_utils import run_bass_kernel_spmd

dt = mybir.dt
AF = mybir.ActivationFunctionType
NCORES = 8
B = 32            # images per core
T = 256           # tokens per image at conv resolution (16x16)
NT = B * T        # 8192
NS = 64           # random samples per image
K = 9             # nearest neighbors

L1_DT = dt.float32    # layer-1 conv-branch dtype (exact: feeds layer-2 KNN)
L2_DT = dt.bfloat16   # layer-2 conv-branch dtype (f32r broken on HW)


def _bf16(x):
    import ml_dtypes
    return np.asarray(x, np.float32).astype(ml_dtypes.bfloat16)


def prep_weights(w):
    """Core-independent input tensors (weights, indices)."""
    m = {}
    def samp_idx(idx, groups):
        t = np.zeros((16 * groups, 128), np.int16)
        s = np.arange(128)
        for g in range(groups):
            for p in range(16):
                t[16 * g + p, :] = (s // 4) * 256 + idx[16 * (s % 4) + p]
        return t
    m['idxs1'] = samp_idx(np.asarray(w['idx1']), 8)
    m['idxs2'] = samp_idx(np.asarray(w['idx2']), 4)

    wc1 = np.zeros((96, 48), np.float32)
    for dh in range(3):
        for dw in range(3):
            wc1[dh * 32:dh * 32 + 12, dw * 16:(dw + 1) * 16] = w['w1a'][:, :, dh, dw].T
    m['wconv1'] = wc1
    wy1a = np.zeros((128, 16), np.float32)
    for k in range(8):
        wy1a[16 * k:16 * k + 12] = w['w1b'][:, :, k].T
    m['wy2_1a'] = wy1a
    wy1b = np.zeros((16, 16), np.float32)
    wy1b[:12] = w['w1b'][:, :, 8].T
    m['wy2_1b'] = wy1b
    w1p = w['w1p'][:, :, 0, 0]
    wp1 = np.zeros((64, 64), np.float32)
    wp1[0:16] = w1p[:, 0:16].T
    wp1[32:48] = w1p[:, 16:32].T
    m['w1p_l'] = wp1
    wc2 = np.zeros((64, 288), np.float32)
    for dh in range(3):
        for dw in range(3):
            wc2[:, (dh * 3 + dw) * 32:(dh * 3 + dw) * 32 + 32] = w['w2a'][:, :, dh, dw].T
    m['wconv2'] = _bf16(wc2)
    wy2 = np.zeros((64, 288), np.float32)
    for k in range(K):
        wy2[:, k * 32:(k + 1) * 32] = w['w2b'][:, :, k].T
    m['wy2_2'] = _bf16(wy2)
    w2p = w['w2p'][:, :, 0, 0]
    m['w2p_l'] = _bf16(np.ascontiguousarray(w2p.T))
    m['b1a'] = np.asarray(w['b1a']).reshape(16, 1).astype(np.float32)
    m['b1b'] = np.asarray(w['b1b']).reshape(16, 1).astype(np.float32)
    m['b1p'] = np.asarray(w['b1p']).reshape(64, 1).astype(np.float32)
    m['b2a'] = np.asarray(w['b2a']).reshape(32, 1).astype(np.float32)
    m['b2b'] = np.asarray(w['b2b']).reshape(32, 1).astype(np.float32)
    m['b2p'] = np.asarray(w['b2p']).reshape(128, 1).astype(np.float32)
    m['onesc1'] = np.concatenate([np.ones((12, 1)), np.zeros((4, 1))]).astype(np.float32)
    m['onesc2'] = np.ones((64, 1), np.float32)
    m['neghalf'] = np.full((1, 128), -0.5, np.float32)
    # FC1 weights per spatial position, bf16, 4-pos tiles (64, 128, 4096)
    fc1 = np.asarray(w['fc1w']).reshape(1024, 32, 32, 32)   # f, cc, hh, ww
    fc1 = fc1.reshape(1024, 32, 16, 2, 16, 2)               # f, cc, h, i, w, j
    fc1 = fc1.transpose(2, 4, 1, 3, 5, 0)                   # h, w, cc, i, j, f
    fc1 = fc1.reshape(256, 128, 1024)                       # pos, ch, f
    m['fc1wp'] = _bf16(fc1.reshape(64, 4, 128, 1024).transpose(0, 2, 1, 3)
                       .reshape(64, 128, 4096))
    m['fc1bt'] = np.asarray(w['fc1b']).reshape(8, 128, 1).astype(np.float32)
    m['fc2w_l'] = np.ascontiguousarray(np.asarray(w['fc2w']).T).astype(np.float32)
    m['fc2b'] = np.asarray(w['fc2b']).reshape(10, 1).astype(np.float32)
    m['ident'] = np.eye(16, dtype=np.float32)
    return m


def prep_x(x_core):
    """(32,3,32,32) -> xun (16, NT) with ones row at 12."""
    xu = x_core.reshape(B, 3, 16, 2, 16, 2).transpose(1, 3, 5, 0, 2, 4)
    xu = np.ascontiguousarray(xu).reshape(12, NT).astype(np.float32)
    xun = np.zeros((16, NT), np.float32)
    xun[:12] = xu
    return xun


def build_bass(stage=3):
    F32 = dt.float32
    nc = bacc.Bacc("TRN2", target_bir_lowering=False, debug=False)
    F32, I16, U16, BF16 = dt.float32, dt.int16, dt.uint16, dt.bfloat16

    def din(name, shape, d=F32):
        return nc.dram_tensor(name, shape, d, kind="ExternalInput")

    xun_d = din('xun', [16, NT])
    idxs1_d = din('idxs1', [128, 128], I16)
    idxs2_d = din('idxs2', [64, 128], I16)
    wconv1_d = din('wconv1', [96, 48], L1_DT)
    wy2_1a_d = din('wy2_1a', [128, 16], L1_DT)
    wy2_1b_d = din('wy2_1b', [16, 16], L1_DT)
    w1p_l_d = din('w1p_l', [64, 64], L1_DT)
    wconv2_d = din('wconv2', [64, 288], L2_DT)
    wy2_2_d = din('wy2_2', [64, 288], L2_DT)
    w2p_l_d = din('w2p_l', [64, 128], L2_DT)
    b1a_d, b1b_d, b1p_d = din('b1a', [16, 1]), din('b1b', [16, 1]), din('b1p', [64, 1])
    b2a_d, b2b_d, b2p_d = din('b2a', [32, 1]), din('b2b', [32, 1]), din('b2p', [128, 1])
    onesc1_d, onesc2_d = din('onesc1', [16, 1]), din('onesc2', [64, 1])
    neghalf_d = din('neghalf', [1, 128])
    fc1wp_d = din('fc1wp', [64, 128, 4096], BF16)
    fc1bt_d = din('fc1bt', [8, 128, 1])
    fc2w_l_d = din('fc2w_l', [1024, 10])
    fc2b_d = din('fc2b', [10, 1])
    ident_d = din('ident', [16, 16])
    if stage >= 3:
        out_d = nc.dram_tensor('out', [B, 10], F32, kind="ExternalOutput")
    if stage == 1:
        h2_d = nc.dram_tensor('h2_dbg', [64, NT], F32, kind="ExternalOutput")
        nbr_d = nc.dram_tensor('nbr_dbg', [128, 576], dt.uint16, kind="ExternalOutput")
    if stage == 2:
        yp2_d = nc.dram_tensor('yp2_dbg', [128, NT], dt.bfloat16, kind="ExternalOutput")
    if stage == 4:
        hfc_d = nc.dram_tensor('hfc_dbg', [128, 256], F32, kind="ExternalOutput")
        fw_d = nc.dram_tensor('fw_dbg', [128, 4096], dt.bfloat16, kind="ExternalOutput")
        yp2b_d = nc.dram_tensor('yp2b_dbg', [128, NT], dt.bfloat16, kind="ExternalOutput")

    with TileContext(nc) as tc:
        with tc.tile_pool(name="consts", bufs=1) as cpool, \
             tc.tile_pool(name="big", bufs=1) as big, \
             tc.tile_pool(name="work", bufs=2) as work, \
             tc.tile_pool(name="small", bufs=1) as small, \
             tc.tile_pool(name="gp", bufs=1) as gp, \
             tc.tile_pool(name="fcw", bufs=2) as fcw, \
             tc.tile_pool(name="dram", bufs=1, space="DRAM") as dram:

            def load(dram_t, name):
                t = cpool.tile(list(dram_t.shape), dram_t.dtype, name=name)
                nc.scalar.dma_start(t[:], dram_t[:])
                return t

            wconv1 = load(wconv1_d, 'wconv1')
            wy2_1a = load(wy2_1a_d, 'wy2_1a')
            wy2_1b = load(wy2_1b_d, 'wy2_1b')
            w1p_l = load(w1p_l_d, 'w1p_l')
            wconv2 = load(wconv2_d, 'wconv2')
            wy2_2 = load(wy2_2_d, 'wy2_2')
            w2p_l = load(w2p_l_d, 'w2p_l')
            b1a, b1b, b1p = load(b1a_d, 'b1a'), load(b1b_d, 'b1b'), load(b1p_d, 'b1p')
            b2a, b2b, b2p = load(b2a_d, 'b2a'), load(b2b_d, 'b2b'), load(b2p_d, 'b2p')
            onesc1, onesc2 = load(onesc1_d, 'onesc1'), load(onesc2_d, 'onesc2')
            neghalf = load(neghalf_d, 'neghalf')
            fc2b = load(fc2b_d, 'fc2b')
            fc1bt = []
            for r in range(8):
                bt = cpool.tile([128, 1], F32, name=f'fc1bt{r}')
                nc.scalar.dma_start(bt[:], fc1bt_d[r])
                fc1bt.append(bt)
            ident = load(ident_d, 'ident')
            idxs1, idxs2 = load(idxs1_d, 'idxs1'), load(idxs2_d, 'idxs2')
            fc2w = cpool.tile([128, 80], F32, name='fc2w')
            for r in range(8):
                nc.scalar.dma_start(fc2w[:, r * 10:(r + 1) * 10],
                                    fc2w_l_d[r * 128:(r + 1) * 128, :])

            # xun replicated to all 8 gpsimd groups
            xun = big.tile([128, NT], F32, tag="act", name='xun')
            for g in range(8):
                nc.sync.dma_start(xun[16 * g:16 * (g + 1), :], xun_d[:])

            # FC1 weight stream (program-order early; consumed at the end)
            fc1w_tiles = []
            for i in range(64 if stage >= 3 else 0):
                ft = fcw.tile([128, 4096], BF16, tag="fc1w", name=f'fc1w{i}')
                nc.sync.dma_start(ft[:], fc1wp_d[i])
                fc1w_tiles.append(ft)

            d3_1 = dram.tile([K, 128, 64], I16, name='d3_1')
            d3_2 = dram.tile([K, 128, 64], I16, name='d3_2')

            with tc.tile_pool(name="ps", bufs=2, space="PSUM") as ps, \
                 tc.tile_pool(name="psd", bufs=2, space="PSUM") as psd:

                # ================== shared KNN machinery ==================
                def knn_topk(lay, src, ctr, samp, snf, d3):
                    """dist scores + top-9 -> gather-index DRAM staging."""
                    ones_t = onesc1 if lay == 1 else onesc2
                    nch = 16 if lay == 1 else 64
                    for c4 in range(4):
                        sq = work.tile([nch, 512], F32, tag="sq", name=f'sq{lay}_{c4}')
                        nc.vector.tensor_mul(sq[:], samp[0:nch, c4 * 512:(c4 + 1) * 512],
                                             samp[0:nch, c4 * 512:(c4 + 1) * 512])
                        pssn = psd.tile([1, 512], F32, tag="sn", name=f'sn{lay}_{c4}')
                        nc.tensor.matmul(pssn[:], ones_t[:], sq[:], start=True, stop=True)
                        nc.scalar.copy(snf[:, c4 * 512:(c4 + 1) * 512], pssn[:])
                    nbr = small.tile([128, 576], U16, tag="nbr", name=f'nbr{lay}')
                    for b in range(B):
                        for h in range(2):
                            bh = b * 2 + h
                            pd = psd.tile([128, NS], F32, tag="dist", name=f'd{lay}_{bh}')
                            nc.tensor.matmul(pd[:], src[0:ctr, bh * 128:(bh + 1) * 128],
                                             samp[0:ctr, b * NS:(b + 1) * NS],
                                             start=True, stop=False)
                            nc.tensor.matmul(pd[:], neghalf[0:1, :],
                                             snf[:, b * NS:(b + 1) * NS],
                                             start=False, stop=True)
                            ndt = work.tile([128, NS], F32, tag="ndt", name=f'nd{lay}_{bh}')
                            nc.scalar.copy(ndt[:], pd[:])
                            mx8 = work.tile([128, 8], F32, tag="mx8", name=f'm8_{lay}_{bh}')
                            nc.vector.max(mx8[:], ndt[:])
                            nc.vector.max_index(nbr[:, bh * 9:bh * 9 + 8], mx8[:], ndt[:])
                            nd2 = work.tile([128, NS], F32, tag="nd2", name=f'n2_{lay}_{bh}')
                            nc.vector.match_replace(nd2[:], mx8[:], ndt[:], -1e30)
                            mx9 = work.tile([128, 8], F32, tag="mx9", name=f'm9_{lay}_{bh}')
                            nc.vector.max(mx9[:], nd2[:])
                            i9 = work.tile([128, 8], U16, tag="i9", name=f'i9_{lay}_{bh}')
                            nc.vector.max_index(i9[:], mx9[:], nd2[:])
                            nc.vector.tensor_copy(nbr[:, bh * 9 + 8:bh * 9 + 9], i9[:, 0:1])
                    iot = small.tile([128, 576], I16, tag="iot", name=f'iot{lay}')
                    nc.gpsimd.iota(iot[:], pattern=[[64, 32], [0, 2], [0, 9]],
                                   base=0, channel_multiplier=0)
                    nbi = small.tile([128, 576], I16, tag="nbi", name=f'nbi{lay}')
                    nc.vector.tensor_copy(nbi[:], nbr[:])
                    isrc = small.tile([128, 576], I16, tag="isrc", name=f'isrc{lay}')
                    nc.vector.tensor_add(isrc[:], nbi[:], iot[:])
                    iview = isrc[:].rearrange("p (bh k) -> p k bh", k=K)
                    for k in range(K):
                        nc.scalar.dma_start(d3[k], iview[:, k, :])
                    return nbr

                def read_idx(dst16, d3, k):
                    """(16, 512) wrapped idx rows from d3[k]; dst16 at any base."""
                    d3v = d3[:].rearrange("k (a p) bh -> k a p bh", p=16)
                    src = d3v[k, :, :, :].rearrange("a p bh -> p a bh")
                    nc.scalar.dma_start(dst16.rearrange("p (a bh) -> p a bh", bh=64), src)

                # ======================= LAYER 1 =======================
                samp1 = big.tile([128, 2048], F32, tag="samp", name='samp1')
                nc.gpsimd.ap_gather(samp1[:], xun[:], idxs1[:],
                                    channels=128, num_elems=NT, d=1, num_idxs=2048)
                snf1 = small.tile([1, 2048], F32, tag="snf", name='snf1')
                nbr1_keep = knn_topk(1, xun, 12, samp1, snf1, d3_1)

                # conv1: dh-stacked zero-padded tile, processed in image halves
                ycat1 = big.tile([64, NT], L1_DT, tag="cat", name='ycat1')
                nc.gpsimd.memset(ycat1[:], 0.0)
                HB = B // 2
                xsrc = xun[0:12, :].rearrange("c (b h w) -> c b h w", b=B, h=16, w=16)
                for bhf in range(2):
                    xpad1 = big.tile([96, HB * 324], L1_DT, tag="padbuf",
                                     name=f'xpad1_{bhf}')
                    nc.gpsimd.memset(xpad1[:], 0.0)
                    xpv = xpad1[:].rearrange("p (b hh ww) -> p b hh ww",
                                             b=HB, hh=18, ww=18)
                    xs = xsrc[:, bhf * HB:(bhf + 1) * HB]
                    # block dh holds x at row (h + 1 - dh) so matmul reads rows 0:16
                    nc.gpsimd.tensor_copy(xpv[0:12, :, 1:17, 1:17], xs)
                    nc.gpsimd.tensor_copy(xpv[32:44, :, 0:16, 1:17], xs)
                    nc.gpsimd.tensor_copy(xpv[64:76, :, 0:15, 1:17], xs[:, :, 1:16, :])
                    for ch in range(8):
                        b0 = ch * 2
                        cho = bhf * 8 + ch
                        p1 = ps.tile([16, 512], F32, tag="yps", name=f'c1_{cho}')
                        for dw in range(3):
                            rhs = xpv[:, b0:b0 + 2, 0:16, dw:dw + 16]
                            nc.tensor.matmul(
                                p1[:].rearrange("o (b h w) -> o b h w", b=2, h=16, w=16),
                                wconv1[:, dw * 16:(dw + 1) * 16], rhs,
                                start=(dw == 0), stop=(dw == 2))
                        nc.scalar.activation(ycat1[0:16, cho * 512:(cho + 1) * 512],
                                             p1[:], AF.Relu, bias=b1a[:])

                idxg1 = small.tile([128, 512], I16, name='idxg1')
                idxg1b = small.tile([16, 512], I16, name='idxg1b')
                for g in range(8):
                    read_idx(idxg1[16 * g:16 * g + 16, :], d3_1, g)
                read_idx(idxg1b[:, :], d3_1, 8)
                for je in range(8):
                    g1 = gp.tile([128, 1024], L1_DT, tag="g_0", name=f'g1_{je}')
                    g1b = gp.tile([16, 1024], L1_DT, tag="g_1", name=f'g1b_{je}')
                    nc.gpsimd.ap_gather(g1[:], samp1[:], idxg1[:, je * 64:(je + 1) * 64],
                                        channels=128, num_elems=2048, d=1, num_idxs=1024)
                    nc.gpsimd.ap_gather(g1b[:], samp1[0:16, :],
                                        idxg1b[:, je * 64:(je + 1) * 64],
                                        channels=16, num_elems=2048, d=1, num_idxs=1024)
                    for chk in range(2):
                        p2 = ps.tile([16, 512], F32, tag="yps", name=f'y1_{je}_{chk}')
                        nc.tensor.matmul(p2[:], wy2_1a[:],
                                         g1[:, chk * 512:(chk + 1) * 512],
                                         start=True, stop=False)
                        nc.tensor.matmul(p2[:], wy2_1b[:],
                                         g1b[:, chk * 512:(chk + 1) * 512],
                                         start=False, stop=True)
                        bh0 = chk * 32
                        wv = ycat1[32:48, :].rearrange("p (bh t) -> p bh t", t=128)
                        nc.scalar.activation(
                            wv[:, bh0:bh0 + 32, 16 * je:16 * je + 16],
                            p2[:].rearrange("o (bh p) -> o bh p", p=16),
                            AF.Relu, bias=b1b[:])

                h2 = big.tile([64, NT], F32, tag="act", name='h2')
                for ch in range(16):
                    p3 = ps.tile([64, 512], F32, tag="yps2", name=f'p1_{ch}')
                    nc.tensor.matmul(p3[:], w1p_l[:], ycat1[:, ch * 512:(ch + 1) * 512],
                                     start=True, stop=True)
                    nc.scalar.activation(h2[:, ch * 512:(ch + 1) * 512], p3[:],
                                         AF.Identity, bias=b1p[:])

                if stage == 1:
                    nc.sync.dma_start(h2_d[:], h2[:])
                    nc.sync.dma_start(nbr_d[:], nbr1_keep[:])
                # ======================= LAYER 2 =======================
                if stage >= 2:
                    samp2 = big.tile([64, 2048], F32, tag="samp", name='samp2')
                    nc.gpsimd.ap_gather(samp2[:], h2[:], idxs2[:],
                                        channels=64, num_elems=NT, d=1, num_idxs=2048)
                    snf2 = small.tile([1, 2048], F32, tag="snf", name='snf2')
                    knn_topk(2, h2, 64, samp2, snf2, d3_2)

                    ycat2 = big.tile([64, NT], L2_DT, tag="cat", name='ycat2')
                    nc.gpsimd.memset(ycat2[:], 0.0)
                    h2src = h2[:].rearrange("c (b h w) -> c b h w", b=B, h=16, w=16)
                    for bhf in range(2):
                        h2pad = big.tile([64, HB * 324], L2_DT, tag="padbuf",
                                         name=f'h2pad_{bhf}')
                        nc.gpsimd.memset(h2pad[:], 0.0)
                        h2pv = h2pad[:].rearrange("p (b hh ww) -> p b hh ww",
                                                  b=HB, hh=18, ww=18)
                        nc.gpsimd.tensor_copy(h2pv[:, :, 1:17, 1:17],
                                              h2src[:, bhf * HB:(bhf + 1) * HB])
                        for ch in range(8):
                            b0 = ch * 2
                            cho = bhf * 8 + ch
                            p4 = ps.tile([32, 512], F32, tag="yps", name=f'c2_{cho}')
                            for dh in range(3):
                                for dw in range(3):
                                    rhs = h2pv[:, b0:b0 + 2, dh:dh + 16, dw:dw + 16]
                                    nc.tensor.matmul(
                                        p4[:].rearrange("o (b h w) -> o b h w",
                                                        b=2, h=16, w=16),
                                        wconv2[:, (dh * 3 + dw) * 32:(dh * 3 + dw) * 32 + 32],
                                        rhs, start=(dh == 0 and dw == 0),
                                        stop=(dh == 2 and dw == 2))
                            nc.scalar.activation(ycat2[0:32, cho * 512:(cho + 1) * 512],
                                                 p4[:], AF.Relu, bias=b2a[:])

                    samp2r = small.tile([64, 4096], L2_DT, tag="samp2r", name='samp2r')
                    nc.vector.memset(samp2r[:], 0.0)
                    s2v = samp2r[:].rearrange("c (n two) -> c n two", two=2)
                    nc.scalar.copy(s2v[:, :, 0:1],
                                   samp2[:].rearrange("c (n o) -> c n o", o=1))
                    idxg2 = []
                    for k in range(K):
                        t = small.tile([64, 512], I16, name=f'idxg2_{k}')
                        for g in range(4):
                            read_idx(t[16 * g:16 * g + 16, :], d3_2, k)
                        idxg2.append(t)
                    for je in range(8):
                        g2 = []
                        for k in range(K):
                            gt = gp.tile([64, 2048], L2_DT, tag=f"g_{k}", name=f'g2_{k}_{je}')
                            nc.gpsimd.ap_gather(gt[:].rearrange("c (j two) -> c j two", two=2),
                                                samp2r[:].rearrange("c (n two) -> c n two", two=2),
                                                idxg2[k][:, je * 64:(je + 1) * 64],
                                                channels=64, num_elems=2048, d=2,
                                                num_idxs=1024)
                            g2.append(gt)
                        for chk in range(2):
                            p5 = ps.tile([32, 512], F32, tag="yps", name=f'y2_{je}_{chk}')
                            for k in range(K):
                                gv = g2[k][:].rearrange("c (j two) -> c j two", two=2)
                                nc.tensor.matmul(p5[:], wy2_2[:, k * 32:(k + 1) * 32],
                                                 gv[:, chk * 512:(chk + 1) * 512, 0],
                                                 start=(k == 0), stop=(k == 8))
                            bh0 = chk * 32
                            wv = ycat2[32:64, :].rearrange("p (bh t) -> p bh t", t=128)
                            nc.scalar.activation(
                                wv[:, bh0:bh0 + 32, 16 * je:16 * je + 16],
                                p5[:].rearrange("o (bh p) -> o bh p", p=16),
                                AF.Relu, bias=b2b[:])

                    yp2 = big.tile([128, NT], dt.bfloat16, tag="yp2", name='yp2')
                    for ch in range(16):
                        p6 = ps.tile([128, 512], F32, tag="yps2", name=f'p2_{ch}')
                        nc.tensor.matmul(p6[:], w2p_l[:], ycat2[:, ch * 512:(ch + 1) * 512],
                                         start=True, stop=True)
                        nc.scalar.activation(yp2[:, ch * 512:(ch + 1) * 512], p6[:],
                                             AF.Identity, bias=b2p[:])

            if stage == 2:
                nc.sync.dma_start(yp2_d[:], yp2[:])
            if stage >= 3:
                # ======================= FC head =======================
                hfc = small.tile([128, 256], F32, name='hfc')
                with tc.tile_pool(name="psfc", bufs=1, space="PSUM") as psfc:
                    yv = yp2[:].rearrange("c (b pos) -> c pos b", pos=T)
                    fps = [psfc.tile([128, 32], F32, tag=f"fc{i}", name=f'fps{i}')
                           for i in range(8)]
                    for pos in range(T):
                        wt = fc1w_tiles[pos // 4]
                        q = pos % 4
                        for fc in range(8):
                            nc.tensor.matmul(
                                fps[fc][:],
                                wt[:, q * 1024 + fc * 128:q * 1024 + (fc + 1) * 128],
                                yv[:, pos, :], start=(pos == 0), stop=(pos == T - 1))
                    for fc in range(8):
                        nc.scalar.activation(hfc[:, fc * 32:(fc + 1) * 32],
                                             fps[fc][:], AF.Relu,
                                             bias=fc1bt[fc][:])
                    if stage == 4:
                        nc.sync.dma_start(hfc_d[:], hfc[:])
                        nc.sync.dma_start(fw_d[:], fc1w_tiles[0][:])
                        nc.sync.dma_start(yp2b_d[:], yp2[:])
                with tc.tile_pool(name="psf2", bufs=1, space="PSUM") as psf2:
                    p7 = psf2.tile([10, 32], F32, tag="fc2o", name='p7')
                    for r in range(8):
                        nc.tensor.matmul(p7[:], fc2w[:, r * 10:(r + 1) * 10],
                                         hfc[:, r * 32:(r + 1) * 32],
                                         start=(r == 0), stop=(r == 7))
                    yo = small.tile([10, 32], F32, name='yo')
                    nc.scalar.activation(yo[:], p7[:], AF.Identity, bias=fc2b[:])
                    pt = psf2.tile([32, 10], F32, tag="tr", name='pt')
                    nc.tensor.transpose(pt[:], yo[:], ident[0:10, 0:10])
                    yout = small.tile([32, 10], F32, name='yout')
                    nc.scalar.copy(yout[:], pt[:])
                    nc.sync.dma_start(out_d[:], yout[:])
    nc.compile()
    return nc


_NC = None


def kernel(**inputs):
    global _NC
    x = np.asarray(inputs['x'], np.float32)
    if _NC is None:
        _NC = build_bass()
    wmap = prep_weights(inputs)
    in_maps = []
    for c in range(NCORES):
        m = dict(wmap)
        m['xun'] = prep_x(x[c * B:(c + 1) * B])
        in_maps.append(m)
    res = run_bass_kernel_spmd(_NC, in_maps, core_ids=list(range(NCORES)))
    return np.concatenate([res.results[c]['out'] for c in range(NCORES)], axis=0)



# revision 20
# speedup vs baseline: 3.1865x; 3.1865x over previous
"""Trainium2 Bass kernel for nn_B_Conv2d_ConvNN_K_N (retrieval_knn).

Data-parallel over 8 NeuronCores: 32 images/core, weights replicated.

KNN neighbor aggregation is reformulated as a one-hot matmul: per image,
P_k = (W_k @ samp)^T, and a rank-k membership mask A_k[t, n] =
(score[t,n] == kth_max[t]) / count, so y[o,t] = sum_k (P_k^T A_k^T)[o,t].
Count normalization makes duplicated sample columns exact.  This removes
all per-token gathers and the DRAM index staging of the previous version.

Convs use unpadded clipped-tap matmuls (border taps write PSUM subranges).
FC1 keeps activations stationary (LDW [128,32] per position) and streams
the 64MB weight through a deep-prefetch tile ring.
"""
import sys
if '/opt/trn_rl_repo' not in sys.path:
    sys.path.insert(0, '/opt/trn_rl_repo')

import numpy as np
import concourse.bacc as bacc
import concourse.mybir as mybir
from concourse.tile import TileContext
from concourse.bass_utils import run_bass_kernel_spmd

dt = mybir.dt
AF = mybir.ActivationFunctionType
ALU = mybir.AluOpType
NCORES = 8
B = 32            # images per core
T = 256           # tokens per image at conv resolution (16x16)
NT = B * T        # 8192
NS = 64           # random samples per image
K = 9             # nearest neighbors
NFCW = 9          # fc1 weight tiles in flight (8KB free-bytes each)


def _bf16(x):
    import ml_dtypes
    return np.asarray(x, np.float32).astype(ml_dtypes.bfloat16)


def prep_weights(w):
    """Core-independent input tensors (weights, indices)."""
    m = {}

    def samp_idx(idx, groups):
        t = np.zeros((16 * groups, 128), np.int16)
        s = np.arange(128)
        for g in range(groups):
            for p in range(16):
                t[16 * g + p, :] = (s // 4) * 256 + idx[16 * (s % 4) + p]
        return t
    m['idxs2'] = samp_idx(np.asarray(w['idx2']), 4)

    w1a = np.asarray(w['w1a'], np.float32)       # (16, 12, 3, 3)
    wc1 = np.zeros((12, 144), np.float32)
    for dh in range(3):
        for dw in range(3):
            tap = dh * 3 + dw
            wc1[:, tap * 16:(tap + 1) * 16] = w1a[:, :, dh, dw].T
    m['wc1'] = wc1
    w1b = np.asarray(w['w1b'], np.float32)       # (16, 12, 9)
    w1bkT = np.zeros((16, 144), np.float32)
    for k in range(K):
        w1bkT[0:12, k * 16:(k + 1) * 16] = w1b[:, :, k].T
    m['w1bkT'] = w1bkT
    w1p = np.asarray(w['w1p'], np.float32)[:, :, 0, 0]   # (64, 32)
    w1pl = np.zeros((48, 64), np.float32)
    w1pl[0:16] = w1p[:, 0:16].T
    w1pl[32:48] = w1p[:, 16:32].T
    m['w1p_l'] = w1pl

    w2a = np.asarray(w['w2a'], np.float32)       # (32, 64, 3, 3)
    wc2 = np.zeros((64, 288), np.float32)
    for dh in range(3):
        for dw in range(3):
            tap = dh * 3 + dw
            wc2[:, tap * 32:(tap + 1) * 32] = w2a[:, :, dh, dw].T
    m['wconv2'] = wc2
    w2b = np.asarray(w['w2b'], np.float32)       # (32, 64, 9)
    w2bkT = np.zeros((64, 288), np.float32)
    for k in range(K):
        w2bkT[:, k * 32:(k + 1) * 32] = w2b[:, :, k].T
    m['w2bkT'] = w2bkT
    w2p = np.asarray(w['w2p'], np.float32)[:, :, 0, 0]   # (128, 64)
    m['w2p_l'] = np.ascontiguousarray(w2p.T)             # (64, 128)

    m['b1a'] = np.asarray(w['b1a']).reshape(16, 1).astype(np.float32)
    m['b1b'] = np.asarray(w['b1b']).reshape(16, 1).astype(np.float32)
    m['b1p'] = np.asarray(w['b1p']).reshape(64, 1).astype(np.float32)
    m['b2a'] = np.asarray(w['b2a']).reshape(32, 1).astype(np.float32)
    m['b2b'] = np.asarray(w['b2b']).reshape(32, 1).astype(np.float32)
    m['b2p'] = np.asarray(w['b2p']).reshape(128, 1).astype(np.float32)
    m['onesc2'] = np.ones((64, 1), np.float32)
    m['ones1'] = np.ones((1, 32), np.float32)
    m['neghalf'] = np.full((1, 128), -0.5, np.float32)
    m['ident128'] = np.eye(128, dtype=np.float32)

    # FC1 weights per spatial position, bf16, 4-pos tiles (64, 128, 4096)
    fc1 = np.asarray(w['fc1w']).reshape(1024, 32, 32, 32)   # f, cc, hh, ww
    fc1 = fc1.reshape(1024, 32, 16, 2, 16, 2)               # f, cc, h, i, w, j
    fc1 = fc1.transpose(2, 4, 1, 3, 5, 0)                   # h, w, cc, i, j, f
    fc1 = fc1.reshape(256, 128, 1024)                       # pos, ch, f
    m['fc1wp'] = _bf16(fc1.reshape(64, 4, 128, 1024).transpose(0, 2, 1, 3)
                       .reshape(64, 128, 4096))
    m['fc1b2'] = np.asarray(w['fc1b']).reshape(1, 1024).astype(np.float32)
    m['fc2w_l'] = np.ascontiguousarray(np.asarray(w['fc2w']).T).astype(np.float32)
    m['fc2b'] = np.asarray(w['fc2b']).reshape(10, 1).astype(np.float32)
    return m


def prep_x(x_core, idx1):
    """Per-core tensors: unshuffled x, host-gathered samples, sample norms."""
    xu = x_core.reshape(B, 3, 16, 2, 16, 2).transpose(1, 3, 5, 0, 2, 4)
    xu = np.ascontiguousarray(xu).reshape(12, NT).astype(np.float32)
    xun = np.zeros((16, NT), np.float32)
    xun[:12] = xu
    samp = xu.reshape(12, B, T)[:, :, idx1]        # (12, B, 64)
    samp1 = np.zeros((16, B * NS), np.float32)
    samp1[:12] = samp.reshape(12, B * NS)
    snf1 = (samp1[:12] ** 2).sum(axis=0).reshape(1, 2048)
    return {'xun': xun, 'samp1': samp1, 'snf1': snf1}


def prep_core_maps(inputs):
    x = np.asarray(inputs['x'], np.float32)
    idx1 = np.asarray(inputs['idx1'])
    wmap = prep_weights(inputs)
    maps = []
    for c in range(NCORES):
        m = dict(wmap)
        m.update(prep_x(x[c * B:(c + 1) * B], idx1))
        maps.append(m)
    return maps


def build_bass(stage=3):
    F32, I16, BF16 = dt.float32, dt.int16, dt.bfloat16
    nc = bacc.Bacc("TRN2", target_bir_lowering=False, debug=False)

    def din(name, shape, d=F32):
        return nc.dram_tensor(name, shape, d, kind="ExternalInput")

    xun_d = din('xun', [16, NT])
    samp1_d = din('samp1', [16, 2048])
    snf1_d = din('snf1', [1, 2048])
    idxs2_d = din('idxs2', [64, 128], I16)
    wc1_d = din('wc1', [12, 144])
    w1bkT_d = din('w1bkT', [16, 144])
    w1p_l_d = din('w1p_l', [48, 64])
    wconv2_d = din('wconv2', [64, 288])
    w2bkT_d = din('w2bkT', [64, 288])
    w2p_l_d = din('w2p_l', [64, 128])
    b1a_d, b1b_d, b1p_d = din('b1a', [16, 1]), din('b1b', [16, 1]), din('b1p', [64, 1])
    b2a_d, b2b_d, b2p_d = din('b2a', [32, 1]), din('b2b', [32, 1]), din('b2p', [128, 1])
    onesc2_d = din('onesc2', [64, 1])
    ones1_d = din('ones1', [1, 32])
    neghalf_d = din('neghalf', [1, 128])
    ident128_d = din('ident128', [128, 128])
    fc1wp_d = din('fc1wp', [64, 128, 4096], BF16)
    fc1b2_d = din('fc1b2', [1, 1024])
    fc2w_l_d = din('fc2w_l', [1024, 10])
    fc2b_d = din('fc2b', [10, 1])
    out_d = nc.dram_tensor('out', [B, 10], F32, kind="ExternalOutput")
    if stage == 1:
        h2_d = nc.dram_tensor('h2_dbg', [64, NT], F32, kind="ExternalOutput")
        yc1_d = nc.dram_tensor('yc1_dbg', [48, NT], F32, kind="ExternalOutput")
    if stage == 2:
        yp2_d = nc.dram_tensor('yp2_dbg', [128, NT], BF16, kind="ExternalOutput")

    with TileContext(nc) as tc:
        with tc.tile_pool(name="consts", bufs=1) as cpool, \
             tc.tile_pool(name="big", bufs=1) as big, \
             tc.tile_pool(name="att", bufs=2) as att, \
             tc.tile_pool(name="work", bufs=2) as work, \
             tc.tile_pool(name="small", bufs=1) as small, \
             tc.tile_pool(name="fcw", bufs=NFCW) as fcw:

            def load(dram_t, name, d=None):
                t = cpool.tile(list(dram_t.shape), d or dram_t.dtype, name=name)
                nc.scalar.dma_start(t[:], dram_t[:])
                return t

            wc1 = load(wc1_d, 'wc1')
            w1bkT = load(w1bkT_d, 'w1bkT')
            w1p_l = load(w1p_l_d, 'w1p_l')
            wconv2 = load(wconv2_d, 'wconv2')
            w2bkT = load(w2bkT_d, 'w2bkT')
            w2p_l = load(w2p_l_d, 'w2p_l')
            b1a, b1b, b1p = load(b1a_d, 'b1a'), load(b1b_d, 'b1b'), load(b1p_d, 'b1p')
            b2a, b2b, b2p = load(b2a_d, 'b2a'), load(b2b_d, 'b2b'), load(b2p_d, 'b2p')
            onesc2 = load(onesc2_d, 'onesc2')
            ones1 = load(ones1_d, 'ones1')
            neghalf = load(neghalf_d, 'neghalf')
            ident128 = load(ident128_d, 'ident128')
            fc1b2 = load(fc1b2_d, 'fc1b2')
            fc2b = load(fc2b_d, 'fc2b')
            idxs2 = load(idxs2_d, 'idxs2')
            samp1 = small.tile([16, 2048], F32, tag="samp", name='samp1')
            nc.scalar.dma_start(samp1[:], samp1_d[:])
            snf1 = small.tile([1, 2048], F32, tag="snf", name='snf1')
            nc.scalar.dma_start(snf1[:], snf1_d[:])
            fc2w = cpool.tile([128, 80], F32, name='fc2w')
            for r in range(8):
                nc.scalar.dma_start(fc2w[:, r * 10:(r + 1) * 10],
                                    fc2w_l_d[r * 128:(r + 1) * 128, :])

            xun = big.tile([16, NT], F32, tag="act", name='xun')
            nc.sync.dma_start(xun[:], xun_d[:])

            # FC1 weight stream (program-order early; consumed at the end)
            fc1w_tiles = []
            for i in range(64):
                ft = fcw.tile([128, 4096], BF16, tag="fc1w", name=f'fc1w{i}')
                nc.sync.dma_start(ft[:], fc1wp_d[i])
                fc1w_tiles.append(ft)

            # Per-image P tiles: rows kn (k*64+n) in 5 chunks of 128, cols o
            Pt1 = cpool.tile([128, B, 5, 16], F32, name='Pt1')
            Pt2 = cpool.tile([128, B, 5, 32], BF16, name='Pt2')

            with tc.tile_pool(name="ps", bufs=2, space="PSUM") as ps:

                def knn_layer(lay, src, ctr, samp, snf, o_ch, wbkT, ycat, bias,
                              adt):
                    """One ConvNN branch: scores -> top9 vals -> masks ->
                    transposed one-hot matmul.  Writes ycat rows [o_ch:2*o_ch]."""
                    Pt = Pt1 if lay == 1 else Pt2
                    # P matmuls, all images
                    for b in range(B):
                        for c in range(5):
                            pP = ps.tile([128, o_ch], F32, tag="pm",
                                         name=f'pP{lay}_{b}_{c}')
                            nk = 1 if c == 4 else 2
                            for kk in range(nk):
                                k = 2 * c + kk
                                nc.tensor.matmul(
                                    pP[64 * kk:64 * kk + 64, :],
                                    samp[0:ctr, b * NS:(b + 1) * NS],
                                    wbkT[0:ctr, k * o_ch:(k + 1) * o_ch],
                                    start=True, stop=True)
                            nc.scalar.copy(Pt[0:64 * nk, b, c, :],
                                           pP[0:64 * nk, :])
                    for b in range(B):
                        AT = att.tile([128, 5, 256], adt, tag="AT",
                                      name=f'AT{lay}_{b}')
                        for h in range(2):
                            bh = b * 2 + h
                            pd = ps.tile([128, NS], F32, tag="pd",
                                         name=f'd{lay}_{bh}')
                            nc.tensor.matmul(pd[:],
                                             src[0:ctr, bh * 128:(bh + 1) * 128],
                                             samp[0:ctr, b * NS:(b + 1) * NS],
                                             start=True, stop=False)
                            nc.tensor.matmul(pd[:], neghalf[0:1, :],
                                             snf[:, b * NS:(b + 1) * NS],
                                             start=False, stop=True)
                            ndt = work.tile([128, NS], F32, tag="ndt",
                                            name=f'nd{lay}_{bh}')
                            nc.scalar.copy(ndt[:], pd[:])
                            mxc = work.tile([128, 9], F32, tag="mxc",
                                            name=f'mxc{lay}_{bh}')
                            nc.vector.max(mxc[:, 0:8], ndt[:])
                            nd2 = work.tile([128, NS], F32, tag="nd2",
                                            name=f'n2_{lay}_{bh}')
                            nc.vector.match_replace(nd2[:], mxc[:, 0:8], ndt[:],
                                                    -1e30)
                            t9 = work.tile([128, 8], F32, tag="t9",
                                           name=f't9_{lay}_{bh}')
                            nc.vector.max(t9[:], nd2[:])
                            nc.vector.tensor_copy(mxc[:, 8:9], t9[:, 0:1])
                            # unnormalized rank masks + counts + scale
                            Au = work.tile([128, 576], F32, tag="Au",
                                           name=f'Au{lay}_{bh}')
                            Auv = Au[:].rearrange("p (k n) -> p k n", n=NS)
                            nc.vector.tensor_tensor(
                                Auv,
                                ndt[:].unsqueeze(1).broadcast_to([128, K, NS]),
                                mxc[:].unsqueeze(2).broadcast_to([128, K, NS]),
                                ALU.is_equal)
                            cnt = work.tile([128, 9], F32, tag="cnt",
                                            name=f'cnt{lay}_{bh}')
                            nc.vector.tensor_reduce(cnt[:], Auv,
                                                    mybir.AxisListType.X,
                                                    ALU.add)
                            rec = work.tile([128, 9], F32, tag="rec",
                                            name=f'rec{lay}_{bh}')
                            nc.vector.reciprocal(rec[:], cnt[:])
                            A = work.tile([128, 576], F32, tag="A",
                                          name=f'A{lay}_{bh}')
                            nc.vector.tensor_tensor(
                                A[:].rearrange("p (k n) -> p k n", n=NS),
                                Auv,
                                rec[:].unsqueeze(2).broadcast_to([128, K, NS]),
                                ALU.mult)
                            # transpose 5 chunks of A into AT
                            for c in range(5):
                                W = 64 if c == 4 else 128
                                ptr = ps.tile([128, 128], F32, tag="ptr",
                                              name=f'tr{lay}_{bh}_{c}')
                                nc.tensor.transpose(
                                    ptr[0:W, :], A[:, c * 128:c * 128 + W],
                                    ident128[:])
                                nc.scalar.copy(AT[0:W, c, h * 128:(h + 1) * 128],
                                               ptr[0:W, :])
                        pm = ps.tile([o_ch, 256], F32, tag="pm",
                                     name=f'pm{lay}_{b}')
                        for c in range(5):
                            W = 64 if c == 4 else 128
                            nc.tensor.matmul(pm[:], Pt[0:W, b, c, :],
                                             AT[0:W, c, :],
                                             start=(c == 0), stop=(c == 4))
                        nc.scalar.activation(
                            ycat[32:32 + o_ch, b * T:(b + 1) * T], pm[:],
                            AF.Relu, bias=bias[:])

                def conv3x3(wtap, cin, o_ch, srcv, ycat, bias):
                    """Unpadded 3x3 conv: clipped tap matmuls per image pair."""
                    for ch in range(16):
                        b0 = ch * 2
                        pc = ps.tile([o_ch, 2, 16, 16], F32, tag="pc",
                                     name=f'c{o_ch}_{ch}')
                        taps = [(1, 1)] + [(dh, dw) for dh in range(3)
                                           for dw in range(3)
                                           if not (dh == 1 and dw == 1)]
                        for i, (dh, dw) in enumerate(taps):
                            tap = dh * 3 + dw
                            hl, hh = max(0, 1 - dh), min(16, 17 - dh)
                            wl, wh = max(0, 1 - dw), min(16, 17 - dw)
                            nc.tensor.matmul(
                                pc[:, :, hl:hh, wl:wh],
                                wtap[:, tap * o_ch:(tap + 1) * o_ch],
                                srcv[0:cin, b0:b0 + 2,
                                     hl + dh - 1:hh + dh - 1,
                                     wl + dw - 1:wh + dw - 1],
                                start=(i == 0), stop=(i == 8))
                        nc.scalar.activation(
                            ycat[0:o_ch, ch * 512:(ch + 1) * 512],
                            pc[:].rearrange("o b h w -> o (b h w)"),
                            AF.Relu, bias=bias[:])

                # ======================= LAYER 1 =======================
                ycat1 = big.tile([48, NT], F32, tag="cat", name='ycat1')
                nc.gpsimd.memset(ycat1[:], 0.0)
                xv = xun[0:12, :].rearrange("c (b h w) -> c b h w", b=B, h=16,
                                            w=16)
                conv3x3(wc1, 12, 16, xv, ycat1, b1a)
                knn_layer(1, xun, 12, samp1, snf1, 16, w1bkT, ycat1, b1b, F32)

                h2 = big.tile([64, NT], F32, tag="act", name='h2')
                for ch in range(16):
                    p3 = ps.tile([64, 512], F32, tag="pc", name=f'p1_{ch}')
                    nc.tensor.matmul(p3[:], w1p_l[:],
                                     ycat1[:, ch * 512:(ch + 1) * 512],
                                     start=True, stop=True)
                    nc.scalar.activation(h2[:, ch * 512:(ch + 1) * 512], p3[:],
                                         AF.Identity, bias=b1p[:])
                if stage == 1:
                    nc.sync.dma_start(h2_d[:], h2[:])
                    nc.sync.dma_start(yc1_d[:], ycat1[:])

                # ======================= LAYER 2 =======================
                samp2 = small.tile([64, 2048], F32, tag="samp", name='samp2')
                nc.gpsimd.ap_gather(samp2[:], h2[:], idxs2[:],
                                    channels=64, num_elems=NT, d=1,
                                    num_idxs=2048)
                snf2 = small.tile([1, 2048], F32, tag="snf", name='snf2')
                for c4 in range(4):
                    sq = work.tile([64, 512], F32, tag="Au", name=f'sq_{c4}')
                    nc.vector.tensor_mul(sq[:],
                                         samp2[:, c4 * 512:(c4 + 1) * 512],
                                         samp2[:, c4 * 512:(c4 + 1) * 512])
                    pssn = ps.tile([1, 512], F32, tag="pd", name=f'sn_{c4}')
                    nc.tensor.matmul(pssn[:], onesc2[:], sq[:],
                                     start=True, stop=True)
                    nc.scalar.copy(snf2[:, c4 * 512:(c4 + 1) * 512], pssn[:])

                ycat2 = big.tile([64, NT], F32, tag="cat", name='ycat2')
                h2v = h2[:].rearrange("c (b h w) -> c b h w", b=B, h=16, w=16)
                conv3x3(wconv2, 64, 32, h2v, ycat2, b2a)
                knn_layer(2, h2, 64, samp2, snf2, 32, w2bkT, ycat2, b2b, BF16)

                # 1x1 conv to 128 ch, pos-major yp2 for the FC head
                yp2 = big.tile([128, NT], BF16, tag="act", name='yp2')
                yp2v = yp2[:].rearrange("c (pos b) -> c b pos", b=B)
                for ch in range(16):
                    p6 = ps.tile([128, 512], F32, tag="pc", name=f'p2_{ch}')
                    nc.tensor.matmul(p6[:], w2p_l[:],
                                     ycat2[:, ch * 512:(ch + 1) * 512],
                                     start=True, stop=True)
                    nc.scalar.activation(
                        yp2v[:, ch * 2:ch * 2 + 2, :],
                        p6[:].rearrange("c (b pos) -> c b pos", b=2),
                        AF.Identity, bias=b2p[:])
                if stage == 2:
                    nc.sync.dma_start(yp2_d[:], yp2[:])

            # ======================= FC head =======================
            with tc.tile_pool(name="psfc", bufs=1, space="PSUM") as psfc, \
                 tc.tile_pool(name="psf2", bufs=2, space="PSUM") as psf2:
                f0 = psfc.tile([32, 512], F32, tag="fc0", name='f0')
                f1 = psfc.tile([32, 512], F32, tag="fc1", name='f1')
                nc.tensor.matmul(f0[:], ones1[:], fc1b2[:, 0:512],
                                 start=True, stop=False)
                nc.tensor.matmul(f1[:], ones1[:], fc1b2[:, 512:1024],
                                 start=True, stop=False)
                for pos in range(T):
                    wt = fc1w_tiles[pos // 4]
                    q = pos % 4
                    lhs = yp2[:, pos * 32:(pos + 1) * 32]
                    nc.tensor.matmul(f0[:], lhs, wt[:, q * 1024:q * 1024 + 512],
                                     start=False, stop=(pos == T - 1))
                    nc.tensor.matmul(f1[:], lhs,
                                     wt[:, q * 1024 + 512:(q + 1) * 1024],
                                     start=False, stop=(pos == T - 1))
                hfcT = small.tile([32, 1024], F32, name='hfcT')
                nc.scalar.activation(hfcT[:, 0:512], f0[:], AF.Relu)
                nc.scalar.activation(hfcT[:, 512:1024], f1[:], AF.Relu)
                hfc2 = small.tile([128, 8, 32], F32, name='hfc2')
                for r in range(8):
                    ptp = psf2.tile([128, 32], F32, tag="tp", name=f'tp{r}')
                    nc.tensor.transpose(ptp[:], hfcT[:, r * 128:(r + 1) * 128],
                                        ident128[0:32, 0:32])
                    nc.scalar.copy(hfc2[:, r, :], ptp[:])
                p7 = psf2.tile([10, 32], F32, tag="fc2o", name='p7')
                for r in range(8):
                    nc.tensor.matmul(p7[:], fc2w[:, r * 10:(r + 1) * 10],
                                     hfc2[:, r, :], start=(r == 0),
                                     stop=(r == 7))
                yo = small.tile([10, 32], F32, name='yo')
                nc.scalar.activation(yo[:], p7[:], AF.Identity, bias=fc2b[:])
                pt = psf2.tile([32, 10], F32, tag="tr", name='pt')
                nc.tensor.transpose(pt[:], yo[:], ident128[0:10, 0:10])
                yout = small.tile([32, 10], F32, name='yout')
                nc.scalar.copy(yout[:], pt[:])
                nc.sync.dma_start(out_d[:], yout[:])
    nc.compile()
    return nc


_NC = None
_NC_STAGE = None


def get_nc(stage=3):
    global _NC, _NC_STAGE
    if _NC is None or _NC_STAGE != stage:
        _NC = build_bass(stage)
        _NC_STAGE = stage
    return _NC


def kernel(**inputs):
    nc = get_nc(3)
    in_maps = prep_core_maps(inputs)
    res = run_bass_kernel_spmd(nc, in_maps, core_ids=list(range(NCORES)))
    return np.concatenate([res.results[c]['out'] for c in range(NCORES)],
                          axis=0)
